# revision 23
# baseline (speedup 1.0000x reference)
"""STCN/STM-style memory read (retrieval_knn) on 8 Trainium2 NeuronCores.

Reference computation (per batch b):
    mk  [64, 8000]  memory keys     (THW = 5*40*40 = 8000)
    mv  [512, 8000] memory values
    qk  [64, 1600]  query keys      (HW = 1600)
    sim = (2 * mk.T @ qk - ||mk||^2) / 8          # [8000, 1600]
    attn = softmax(sim, axis=0)
    out = mv @ attn                                # [512, 1600]

Sharding: 8 cores = 4 batches x 2 query-halves (800 query pixels/core).
Per core the memory axis is padded 8000 -> 8064 = 63 tiles of 128.

Key layout trick: the contraction dim is padded 64 -> 128 (uniform-K matmul
sequences run much faster on the PE), and the spare rows carry ||mk||^2 as
an fp16 hi/lo pair against a constant -0.5 row in qk, so the PE directly
produces dot - 0.5*||mk||^2 and the softmax numerator is just
exp(0.25 * psum) — a single ScalarE activation, no bias input.

On-chip dataflow per core (q processed in 2 chunks of 400, m-tiles in pairs):
    sim_psum[128m, 2*400] = 2x matmul(lhsT=mk_aug[128, 128m], rhs=qk_aug)
    exp[128, 800] (fp16)  = ScalarE Exp(0.25 * sim_psum)     (one op per pair)
    out_psum[128cv, 400] += matmul(lhsT=mvT[128m, 128cv], rhs=exp half)
    acc[128, 400] (fp32)  += exp halves     (VectorE; softmax denominator)
    den[1, 400] = matmul(ones[128,1], acc)            (cross-partition sum)
    recip = 1/den;  bcast[128,400] = matmul(ones[1,128], recip)
    out_sbuf = out_psum * bcast   -> DMA to DRAM  (fp32)
"""

import sys

sys.path.insert(0, "/opt/trn_rl_repo")

import numpy as np

B, CK, CV, T, H, W = 4, 64, 512, 5, 40, 40
THW = T * H * W          # 8000
HW = H * W               # 1600
NT = 63                  # number of 128-row memory tiles after padding
MPAD = NT * 128          # 8064
KDIM = 128               # padded contraction dim (64 keys + 2 aug + zeros)
NCORES = 8
Q = HW // 2              # 800 query pixels per core
QC = 400                 # default query chunk (PSUM bank = 512 fp32 max)
CHUNKS = (400, 400)      # per-core query chunk sizes (sum = Q)
NCV = CV // 128          # 4
PAD_MKSQ = 4.0e4         # ||mk||^2 for padded memory rows -> sim=-5000 -> exp=0

_CACHE = {}
LAST_RESULTS = None      # BassKernelResults of the most recent run (for test.py)


def _build_program(n_reps=1, pairn=2, chunks=CHUNKS, ebufs=4):
    import concourse.bacc as bacc
    import concourse.bass as bass
    import concourse.mybir as mybir
    import concourse.tile as tile
    from concourse.bass import ts

    f16 = mybir.dt.float16
    f32 = mybir.dt.float32
    Exp = mybir.ActivationFunctionType.Exp

    nc = bacc.Bacc(None, target_bir_lowering=False)

    mk_d = nc.dram_tensor("mk", [KDIM, MPAD], f16, kind="ExternalInput")
    qk_d = nc.dram_tensor("qk", [KDIM, Q], f16, kind="ExternalInput")
    mv_d = nc.dram_tensor("mvt", [MPAD, CV], f16, kind="ExternalInput")
    out_d = nc.dram_tensor("out", [CV, Q], f32, kind="ExternalOutput")

    with tile.TileContext(nc) as tc:
        with (
            tc.tile_pool(name="const", bufs=1) as cpool,
            tc.tile_pool(name="keys", bufs=2) as kpool,
            tc.tile_pool(name="mv", bufs=NT + 4) as mvpool,
            tc.tile_pool(name="work", bufs=2) as wpool,
            tc.tile_pool(name="exps", bufs=ebufs) as epool,
            tc.tile_pool(name="osb", bufs=4) as opool,
            tc.tile_pool(name="ps_out", bufs=4, space="PSUM") as ps_out,
            tc.tile_pool(
                name="ps_sim", bufs=(2 if pairn == 2 else 3), space="PSUM"
            ) as ps_sim,
        ):
            ones_col = cpool.tile([128, 1], f32, name="ones_col")
            nc.vector.memset(ones_col[:], 1.0)
            ones_row = cpool.tile([1, 128], f32, name="ones_row")
            nc.vector.memset(ones_row[:], 1.0)

            import contextlib

            loop_ctx = (
                tc.For_i(0, n_reps, 1, hint_engines=(mybir.EngineType.PE,))
                if n_reps > 1
                else contextlib.nullcontext()
            )
            with loop_ctx:
                r = "r0_"
                qk_s = kpool.tile([KDIM, Q], f16, name=r + "qk_s", tag="qk")
                nc.sync.dma_start(qk_s[:], qk_d[:])
                # mk loaded in 16-tile pieces so the first sim matmuls don't
                # wait for the whole 2 MB transfer
                MKSPLIT = 16 * 128
                mk_parts = []
                for j in range((NT + 15) // 16):
                    sz = min(MKSPLIT, MPAD - j * MKSPLIT)
                    mkp = kpool.tile(
                        [KDIM, sz], f16, name=f"{r}mk_s{j}", tag=f"mk{j}"
                    )
                    nc.sync.dma_start(mkp[:], mk_d[:, bass.ds(j * MKSPLIT, sz)])
                    mk_parts.append(mkp)

                mv_tiles = []
                for t in range(NT):
                    mvt = mvpool.tile([128, CV], f16, name=f"{r}mv{t}", tag="mv")
                    nc.sync.dma_start(mvt[:], mv_d[ts(t, 128), :])
                    mv_tiles.append(mvt)

                qoff = 0
                for qc, CSZ in enumerate(chunks):
                    qsl = bass.ds(qoff, CSZ)
                    outs_ps = [
                        ps_out.tile([128, CSZ], f32, name=f"{r}o{qc}_{cv}", tag="out")
                        for cv in range(NCV)
                    ]
                    acc = wpool.tile([128, CSZ], f32, name=f"{r}acc{qc}", tag="acc")

                    # m-tiles processed in pairs: one [128, 800] PSUM tile
                    # holds sim for (t, t+1); a single ScalarE Exp covers
                    # both.  PE computes the next pair's sims while the
                    # scalar engine works, so readouts never stall.
                    # each 512-wide half of the [128, 2, 512] PSUM tile is
                    # bank-aligned; matmuls write cols 0:400 of their half
                    # (a matmul output must not cross a PSUM bank boundary)
                    def make_sim_pair(p):
                        n = min(pairn, NT - p)
                        simp = ps_sim.tile(
                            [128, n, 512], f32, name=f"{r}sim{qc}_{p}", tag="sim"
                        )
                        for i in range(n):
                            t_ = p + i
                            nc.tensor.matmul(
                                simp[:, i, :CSZ],
                                mk_parts[t_ // 16][:, ts(t_ % 16, 128)],
                                qk_s[:, qsl],
                                start=True,
                                stop=True,
                            )
                        return simp, n

                    simp, np_ = make_sim_pair(0)
                    for p in range(0, NT, pairn):
                        cur, ncur = simp, np_
                        expp = epool.tile(
                            [128, ncur, CSZ], f16, name=f"{r}e{qc}_{p}", tag="exp"
                        )
                        nc.scalar.activation(expp[:], cur[:, :, :CSZ], Exp, scale=0.25)
                        if p + pairn < NT:
                            simp, np_ = make_sim_pair(p + pairn)
                        for i in range(ncur):
                            t = p + i
                            esl = expp[:, i, :]
                            if t == 0:
                                nc.vector.tensor_copy(acc[:], esl)
                            else:
                                nc.vector.tensor_add(acc[:], acc[:], esl)
                            for cv in range(NCV):
                                nc.tensor.matmul(
                                    outs_ps[cv][:],
                                    mv_tiles[t][:, ts(cv, 128)],
                                    esl,
                                    start=(t == 0),
                                    stop=(t == NT - 1),
                                )

                    den = ps_sim.tile([1, CSZ], f32, name=f"{r}den{qc}", tag="sim")
                    nc.tensor.matmul(
                        den[:], ones_col[:], acc[:], start=True, stop=True
                    )
                    recip = wpool.tile([1, CSZ], f32, name=f"{r}rcp{qc}", tag="rcp")
                    nc.vector.reciprocal(recip[:], den[:])
                    bc = ps_sim.tile([128, CSZ], f32, name=f"{r}bc{qc}", tag="sim")
                    nc.tensor.matmul(
                        bc[:], ones_row[:], recip[:], start=True, stop=True
                    )
                    bc_sb = wpool.tile([128, CSZ], f32, name=f"{r}bcs{qc}", tag="bcs")
                    nc.scalar.copy(bc_sb[:], bc[:])
                    for cv in range(NCV):
                        o_sb = opool.tile(
                            [128, CSZ], f32, name=f"{r}os{qc}_{cv}", tag="osb"
                        )
                        nc.vector.tensor_mul(o_sb[:], outs_ps[cv][:], bc_sb[:])
                        nc.sync.dma_start(out_d[ts(cv, 128), qsl], o_sb[:])
                    qoff += CSZ

    nc.compile()
    return nc


def _get_program():
    if "nc" not in _CACHE:
        _CACHE["nc"] = _build_program()
    return _CACHE["nc"]


def host_prep(mem_key, mem_val, qry_key):
    """Layout/sharding prep: returns per-core input maps."""
    mem_key = np.asarray(mem_key, dtype=np.float32)
    mem_val = np.asarray(mem_val, dtype=np.float32)
    qry_key = np.asarray(qry_key, dtype=np.float32)

    mk = mem_key.reshape(B, CK, THW)
    mksq = np.einsum("bcm,bcm->bm", mk, mk)                    # [B, THW]

    # mk_aug rows: 0:64 keys, 64 = ||mk||^2 (fp16 hi), 65 = residual (lo),
    # 66:128 zero.  Padded memory columns get mksq=4e4 -> softmax weight 0.
    mk16 = np.zeros((B, KDIM, MPAD), np.float16)
    mk16[:, :CK, :THW] = mk
    mk16[:, CK, :] = PAD_MKSQ
    hi = mksq.astype(np.float16)
    mk16[:, CK, :THW] = hi
    mk16[:, CK + 1, :THW] = (mksq - hi.astype(np.float32)).astype(np.float16)

    qk16 = np.zeros((B, KDIM, HW), np.float16)
    qk16[:, :CK] = qry_key.reshape(B, CK, HW)
    qk16[:, CK : CK + 2] = -0.5

    mvt = np.zeros((B, MPAD, CV), np.float16)
    mvt[:, :THW, :] = mem_val.reshape(B, CV, THW).transpose(0, 2, 1)

    in_maps = []
    for c in range(NCORES):
        b, h = divmod(c, 2)
        in_maps.append(
            {
                "mk": mk16[b],
                "qk": np.ascontiguousarray(qk16[b, :, h * Q : (h + 1) * Q]),
                "mvt": mvt[b],
            }
        )
    return in_maps


def kernel(mem_key, mem_val, qry_key):
    global LAST_RESULTS
    import os

    # this container's axon client has no NTFF hook; the trace path would
    # crash run_bass_kernel_spmd, so force it off
    os.environ["BASS_NEVER_TRACE"] = "1"
    from concourse.bass_utils import run_bass_kernel_spmd

    in_maps = host_prep(mem_key, mem_val, qry_key)
    nc = _get_program()
    LAST_RESULTS = run_bass_kernel_spmd(nc, in_maps, list(range(NCORES)))

    out = np.empty((B, CV, HW), np.float32)
    for c in range(NCORES):
        b, h = divmod(c, 2)
        out[b, :, h * Q : (h + 1) * Q] = LAST_RESULTS.results[c]["out"]
    return out.reshape(B, CV, H, W)
